# revision 22
# baseline (speedup 1.0000x reference)
"""Causal self-attention (B=4, T=2048, D=1024, single head, no scaling) on 8
Trainium2 NeuronCores.

Sharding: core c -> (batch b = c // 2, half h = c % 2).
The pair (2b, 2b+1) shares batch b:
  - K/V projections are split by key halves: core h projects keys
    [h*1024, (h+1)*1024) only, then the halves are exchanged with a
    2-core AllGather (HBM->HBM), eliminating the duplicated projection
    work of the pure batch-sharded layout.
  - Queries are split by interleaved 128-row chunks: core h owns global
    chunks {2j + h}. Slot L = chunks {h, 2+h, 4+h, 6+h} (kc extent 8),
    slot H = {8+h, ...} (extent 16) -- both classes run the same uniform
    program; validity lives in additive bf16 masks (0 / -30000).
  - The interleaving makes per-128-row causal extents nearly tight for
    BOTH classes, so the PV accumulation is trimmed per query group:
    extents {2,4,6,8} (L) and {10,12,14,16} (H) instead of {8,16}.

All matmul inputs are 16-bit (fp16 except exp(S) which needs bf16 range);
PSUM accumulation is fp32. 16-bit stationary halves the LDWEIGHTS pipe
(2 cycles/row vs fp32r's 4) which previously throttled the PE.
Softmax uses a constant bias (-8) instead of a row max; row sums come
from ones-row matmuls + a DRAM-bounce transpose.

Phase order (collectives hidden under compute):
  K-proj(half) -> bounce+AllGather K || V-proj(half) -> bounce+AllGather V
  || Q-proj -> S+exp (reads gathered K) -> row-sums -> PV (reads gathered
  V, trimmed extents) -> 1/l scale -> out.
"""

import os
import numpy as np

import concourse.bass as bass
import concourse.mybir as mybir
import concourse.tile as tile
from concourse import bacc
from concourse.bass_utils import run_bass_kernel_spmd

B, T, D = 4, 2048, 1024
P = 128
NDC = D // P  # 8 contraction chunks over d_model
KH = T // 2  # 1024 keys per core (my half)
QS = 512  # query slot width
NQSUB = QS // P  # 4
SLOT_EXT = (8, 16)  # S key-chunk extent of slot L / slot H
PV_EXT = ((2, 4, 6, 8), (10, 12, 14, 16))  # trimmed PV extents per qsub
NMASK = 16  # mask units: L kc 0..7  +  H kc 8..15
MASK_VAL = -30000.0  # exactly representable in bf16; exp(S+MASK_VAL) == 0
RG = [[0, 1], [2, 3], [4, 5], [6, 7]]  # batch pairs

F32 = mybir.dt.float32
F32R = mybir.dt.float32r
BF16 = mybir.dt.bfloat16
F16 = mybir.dt.float16


def build_nc():
    nc = bacc.Bacc("TRN2", target_bir_lowering=False, debug=False, num_devices=8)

    # x[b].T columns of MY key half / MY query rows (local chunk order)
    xkvT = nc.dram_tensor("xkvT", [D, KH], F16, kind="ExternalInput")
    xqT = nc.dram_tensor("xqT", [D, 2 * QS], F16, kind="ExternalInput")
    wqT = nc.dram_tensor("wqT", [D, D], F16, kind="ExternalInput")  # Wq.T
    wkT = nc.dram_tensor("wkT", [D, D], F16, kind="ExternalInput")
    wvT = nc.dram_tensor("wvT", [D, D], F16, kind="ExternalInput")
    msk = nc.dram_tensor("msk", [NMASK, P, QS], BF16, kind="ExternalInput")
    out = nc.dram_tensor("out", [2 * QS, D], F32, kind="ExternalOutput")

    # collective bounce buffers. K^T half: rows e, cols k (local). V half:
    # rows t (local), cols e. Gather outputs are Shared for HBM-HBM perf.
    kbnc = nc.dram_tensor("kbnc", [D, KH], F16, kind="Internal")
    vbnc = nc.dram_tensor("vbnc", [KH, D], BF16, kind="Internal")
    kg = nc.dram_tensor("kg", [2 * D, KH], F16, kind="Internal")
    vg = nc.dram_tensor("vg", [2 * KH, D], BF16, kind="Internal")

    xkvT_v = xkvT.rearrange("(c p) t -> p c t", p=P)
    xqT_v = xqT.rearrange("(c p) q -> p c q", p=P)
    w_v = {
        "q": wqT.rearrange("(c p) e -> p c e", p=P),
        "k": wkT.rearrange("(c p) e -> p c e", p=P),
        "v": wvT.rearrange("(c p) e -> p c e", p=P),
    }
    kbnc_v = kbnc.rearrange("(c p) k -> p c k", p=P)
    vbnc_v = vbnc.rearrange("(c p) e -> p c e", p=P)
    kg_v = kg.rearrange("(h c p) k -> p h c k", h=2, p=P)
    vg_v = vg.rearrange("(h c p) e -> p h c e", h=2, p=P)

    with tile.TileContext(nc) as tc:
        with (
            tc.tile_pool(name="persist", bufs=1) as persist,
            tc.tile_pool(name="small", bufs=2) as smallp,
            tc.tile_pool(name="dram", bufs=1, space="DRAM") as dramp,
        ):
            pT = persist.tile([P, 24, QS], BF16, tag="pT")  # exp(S^T)  24 KB/p
            ones_f32 = persist.tile([P, 1], F32, tag="ones_f32")
            nc.vector.memset(ones_f32, 1.0)
            ones = persist.tile([P, 1], F32R, tag="ones")
            nc.vector.tensor_copy(out=ones, in_=ones_f32)
            ones_bf = persist.tile([P, 1], BF16, tag="ones_bf")
            nc.vector.tensor_copy(out=ones_bf, in_=ones_f32)
            # exp bias: global constant -C (cancels in the l-normalization)
            negc = persist.tile([P, 1], F32, tag="negc")
            nc.vector.memset(negc, -8.0)
            linv = persist.tile([P, 2, NQSUB], F32, tag="linv")  # 1/l per slot
            warm = persist.tile([P, 2], F32R, tag="warm")
            nc.vector.tensor_copy(out=warm, in_=ones_f32.to_broadcast((P, 2)))

            # HAM warm-up: keep the PE busy while startup DMAs stream in.
            with tc.tile_pool(name="warmps", bufs=1, space="PSUM") as warmps:
                wps = warmps.tile([1, 2], F32)
                for wi in range(56):
                    nc.tensor.matmul(
                        wps, ones, warm, start=(wi == 0), stop=(wi == 55)
                    )

            # xkv tiles live through K and V projections (stationary in V).
            xkvp = tc.alloc_tile_pool(name="xkvp", bufs=16)
            xkv = {}  # (dc, ts) -> tile [P, QS]

            # ============ Phase K: K^T[e, k] for my key half ================
            with (
                tc.tile_pool(name="wkp", bufs=8) as wkp,
                tc.tile_pool(name="kminep", bufs=1) as kminep,
                tc.tile_pool(name="kps", bufs=8, space="PSUM") as kpsp,
            ):
                kmine = kminep.tile([P, NDC, KH], F16, tag="kmine")  # 16 KB/p
                wkc = []
                # first-use order: (wk chunk, x chunk) per dc for slice 0
                for dc in range(NDC):
                    w_t = wkp.tile([P, D], F16, tag="wkc", name=f"wk_{dc}")
                    nc.sync.dma_start(out=w_t, in_=w_v["k"][:, dc, :])
                    wkc.append(w_t)
                    x_t = xkvp.tile([P, QS], F16, tag="xkv", name=f"xkv_{dc}_0")
                    nc.sync.dma_start(out=x_t, in_=xkvT_v[:, dc, 0:QS])
                    xkv[(dc, 0)] = x_t

                for ts in range(2):
                    if ts == 1:
                        for dc in range(NDC):
                            x_t = xkvp.tile(
                                [P, QS], F16, tag="xkv", name=f"xkv_{dc}_1"
                            )
                            nc.sync.dma_start(
                                out=x_t, in_=xkvT_v[:, dc, QS : 2 * QS]
                            )
                            xkv[(dc, 1)] = x_t
                    for half in range(2):
                        pss = [
                            kpsp.tile([P, QS], F32, tag="kps", name=f"kps_{ts}_{half}_{i}")
                            for i in range(4)
                        ]
                        for dc in range(NDC):
                            for ei, ec in enumerate(range(half * 4, half * 4 + 4)):
                                nc.tensor.matmul(
                                    pss[ei],
                                    wkc[dc][:, ec * P : (ec + 1) * P],
                                    xkv[(dc, ts)],
                                    start=(dc == 0),
                                    stop=(dc == NDC - 1),
                                )
                        for ei, ec in enumerate(range(half * 4, half * 4 + 4)):
                            nc.any.tensor_copy(
                                out=kmine[:, ec, ts * QS : (ts + 1) * QS],
                                in_=pss[ei],
                            )

                # bounce my K^T half to DRAM and gather the pair's full K^T
                for ec in range(NDC):
                    nc.sync.dma_start(out=kbnc_v[:, ec, :], in_=kmine[:, ec, :])
                nc.gpsimd.collective_compute(
                    "AllGather",
                    mybir.AluOpType.bypass,
                    replica_groups=RG,
                    ins=[kbnc[:, :]],
                    outs=[kg[:, :]],
                )

            # ============ Phase V: V[t, e] for my key half ==================
            with (
                tc.tile_pool(name="wvp", bufs=8) as wvp,
                tc.tile_pool(name="vminep", bufs=1) as vminep,
                tc.tile_pool(name="vps", bufs=4, space="PSUM") as vpsp,
            ):
                vmine = vminep.tile([P, NDC, D], BF16, tag="vmine")  # 16 KB/p
                wvc = []
                for dc in range(NDC):
                    w_t = wvp.tile([P, D], F16, tag="wvc", name=f"wv_{dc}")
                    nc.sync.dma_start(out=w_t, in_=w_v["v"][:, dc, :])
                    wvc.append(w_t)
                for tc2 in range(NDC):  # my 8 key chunks of 128
                    ts, sub = tc2 // 4, tc2 % 4
                    for es in range(2):
                        ps = vpsp.tile([P, QS], F32, tag="vps")
                        for dc in range(NDC):
                            nc.tensor.matmul(
                                ps,
                                xkv[(dc, ts)][:, sub * P : (sub + 1) * P],
                                wvc[dc][:, es * QS : (es + 1) * QS],
                                start=(dc == 0),
                                stop=(dc == NDC - 1),
                            )
                        nc.any.tensor_copy(
                            out=vmine[:, tc2, es * QS : (es + 1) * QS], in_=ps
                        )
                for tc2 in range(NDC):
                    nc.sync.dma_start(out=vbnc_v[:, tc2, :], in_=vmine[:, tc2, :])
                nc.gpsimd.collective_compute(
                    "AllGather",
                    mybir.AluOpType.bypass,
                    replica_groups=RG,
                    ins=[vbnc[:, :]],
                    outs=[vg[:, :]],
                )

            xkvp.release()

            # ============ Phase Q: Q^T[e, q] ================================
            qTp = tc.alloc_tile_pool(name="qTp", bufs=1)
            qT = qTp.tile([P, NDC, 2 * QS], F16, tag="qT")  # 16 KB/p
            with (
                tc.tile_pool(name="wqp", bufs=8) as wqp,
                tc.tile_pool(name="xqp", bufs=16) as xqp,
                tc.tile_pool(name="qps", bufs=8, space="PSUM") as qpsp,
            ):
                wqc = []
                xqc = {}
                for dc in range(NDC):
                    w_t = wqp.tile([P, D], F16, tag="wqc", name=f"wq_{dc}")
                    nc.sync.dma_start(out=w_t, in_=w_v["q"][:, dc, :])
                    wqc.append(w_t)
                    x_t = xqp.tile([P, QS], F16, tag="xq", name=f"xq_{dc}_0")
                    nc.sync.dma_start(out=x_t, in_=xqT_v[:, dc, 0:QS])
                    xqc[(dc, 0)] = x_t
                for qs in range(2):
                    if qs == 1:
                        for dc in range(NDC):
                            x_t = xqp.tile([P, QS], F16, tag="xq", name=f"xq_{dc}_1")
                            nc.sync.dma_start(
                                out=x_t, in_=xqT_v[:, dc, QS : 2 * QS]
                            )
                            xqc[(dc, 1)] = x_t
                    for half in range(2):
                        pss = [
                            qpsp.tile([P, QS], F32, tag="qps", name=f"qps_{qs}_{half}_{i}")
                            for i in range(4)
                        ]
                        for dc in range(NDC):
                            for ei, ec in enumerate(range(half * 4, half * 4 + 4)):
                                nc.tensor.matmul(
                                    pss[ei],
                                    wqc[dc][:, ec * P : (ec + 1) * P],
                                    xqc[(dc, qs)],
                                    start=(dc == 0),
                                    stop=(dc == NDC - 1),
                                )
                        for ei, ec in enumerate(range(half * 4, half * 4 + 4)):
                            nc.any.tensor_copy(
                                out=qT[:, ec, qs * QS : (qs + 1) * QS], in_=pss[ei]
                            )

            # ============ Phase S: S^T[k, q] + exp, from gathered K =========
            with (
                tc.tile_pool(name="ktp", bufs=4) as ktp,
                tc.tile_pool(name="mask", bufs=3) as maskp,
                tc.tile_pool(name="sps", bufs=6, space="PSUM") as spsp,
                tc.tile_pool(name="lrowp", bufs=2, space="PSUM") as lrowp,
            ):
                for kcg in range(16):  # global key chunk
                    h, kcl = kcg // 8, kcg % 8
                    kt = ktp.tile([P, NDC, P], F16, tag="kt")
                    nc.sync.dma_start(
                        out=kt, in_=kg_v[:, h, :, kcl * P : (kcl + 1) * P]
                    )
                    for slot in range(2):
                        if kcg >= SLOT_EXT[slot]:
                            continue
                        u = kcg if slot == 0 else 8 + kcg
                        sps = spsp.tile([P, QS], F32, tag="sps")
                        for ec in range(NDC):
                            nc.tensor.matmul(
                                sps,
                                kt[:, ec, :],
                                qT[:, ec, slot * QS : (slot + 1) * QS],
                                start=(ec == 0),
                                stop=(ec == NDC - 1),
                            )
                        # mask: L -> msk[kc] 0..7, H -> msk[kc] 8..15;
                        # H kc 0..7 is fully valid for both classes.
                        if (slot == 0) or (kcg >= 8):
                            mt = maskp.tile([P, QS], BF16, tag="mask")
                            nc.sync.dma_start(out=mt, in_=msk[kcg, :, :])
                            nc.vector.tensor_add(out=sps, in0=sps, in1=mt)
                        nc.scalar.activation(
                            out=pT[:, u, :],
                            in_=sps,
                            func=mybir.ActivationFunctionType.Exp,
                            bias=negc[:, :],
                        )

                # Row sums: l[slot, q] = sum_k exp(S^T)[k, q]; DRAM-bounce
                # transpose to column form, reciprocal overlaps PV.
                lrow_d = dramp.tile([2, QS], F32)
                for slot in range(2):
                    ext = SLOT_EXT[slot]
                    lrow_ps = lrowp.tile([1, QS], F32, tag="lrow", name=f"lrow_{slot}")
                    for kc in range(ext):
                        u = kc if slot == 0 else 8 + kc
                        nc.tensor.matmul(
                            lrow_ps,
                            ones_bf,
                            pT[:, u, :],
                            start=(kc == 0),
                            stop=(kc == ext - 1),
                        )
                    lrow_sb = smallp.tile([1, QS], F32, tag="lrow_sb")
                    nc.any.tensor_copy(out=lrow_sb, in_=lrow_ps)
                    # DRAM APs must stay 2-D (1-D APs break NEFF load)
                    nc.sync.dma_start(
                        out=lrow_d[slot : slot + 1, :], in_=lrow_sb[0:1, :]
                    )
                    l_col = smallp.tile([P, NQSUB], F32, tag="lcol")
                    nc.sync.dma_start(
                        out=l_col,
                        in_=lrow_d[slot, :].rearrange("(q p) -> p q", p=P),
                    )
                    nc.vector.reciprocal(out=linv[:, slot, :], in_=l_col)

            qTp.release()

            # ============ Phase PV: O[q, e] from gathered V, trimmed ========
            with (
                tc.tile_pool(name="vsbp", bufs=1) as vsbp,
                tc.tile_pool(name="ostage", bufs=2) as ostagep,
                tc.tile_pool(name="ops", bufs=2, space="PSUM") as opsp,
            ):
                vsb = vsbp.tile([P, 16, D], BF16, tag="vsb")  # 32 KB/p
                for kcg in range(16):
                    h, tcl = kcg // 8, kcg % 8
                    nc.sync.dma_start(
                        out=vsb[:, kcg, :], in_=vg_v[:, h, tcl, :]
                    )

                for slot in range(2):
                    for qsub in range(NQSUB):
                        ext = PV_EXT[slot][qsub]
                        ops = opsp.tile([P, D], F32, tag="o")
                        for kc in range(ext):
                            u = kc if slot == 0 else 8 + kc
                            lhsT = pT[:, u, qsub * P : (qsub + 1) * P]
                            for es in range(2):
                                nc.tensor.matmul(
                                    ops[:, es * QS : (es + 1) * QS],
                                    lhsT,
                                    vsb[:, kc, es * QS : (es + 1) * QS],
                                    start=(kc == 0),
                                    stop=(kc == ext - 1),
                                )
                        o_sb = ostagep.tile([P, D], F32, tag="osb")
                        nc.vector.tensor_scalar_mul(
                            out=o_sb, in0=ops, scalar1=linv[:, slot, qsub : qsub + 1]
                        )
                        r0 = (slot * NQSUB + qsub) * P
                        nc.sync.dma_start(out=out[r0 : r0 + P, :], in_=o_sb)

    nc.compile()
    return nc


_NC_CACHE = []


def _get_nc():
    if not _NC_CACHE:
        _NC_CACHE.append(build_nc())
    return _NC_CACHE[0]


def _global_chunk(cls, slot, j):
    """Global 128-row query chunk for class cls, slot, local chunk j."""
    return slot * 8 + 2 * j + cls


def _build_masks():
    """mask[u, k, q] additive (0 valid / MASK_VAL invalid) per class, bf16.

    Unit u = kc for slot L (0..7), 8 + kc for slot H (8..15).  Query column
    c = j*128 + i maps to global query (slot*8 + 2j + cls)*128 + i.
    """
    import ml_dtypes

    masks = []
    cols = np.arange(QS)
    j, i = cols // P, cols % P
    for cls in range(2):
        m = np.zeros((NMASK, P, QS), np.float32)
        for u in range(NMASK):
            slot = 0 if u < 8 else 1
            kglob = u * P + np.arange(P)[:, None]
            qglob = (_global_chunk(cls, slot, j) * P + i)[None, :]
            m[u] = np.where(kglob <= qglob, 0.0, MASK_VAL)
        masks.append(m.astype(ml_dtypes.bfloat16))
    return masks


def kernel(x, Wq, Wk, Wv):
    x = np.ascontiguousarray(np.asarray(x), dtype=np.float32)
    nc = _get_nc()
    masks = _build_masks()
    wqT = np.ascontiguousarray(np.asarray(Wq, np.float32).T.astype(np.float16))
    wkT = np.ascontiguousarray(np.asarray(Wk, np.float32).T.astype(np.float16))
    wvT = np.ascontiguousarray(np.asarray(Wv, np.float32).T.astype(np.float16))

    in_maps = []
    for c in range(8):
        b, cls = c // 2, c % 2
        xkvT = np.ascontiguousarray(
            x[b][cls * KH : (cls + 1) * KH].T.astype(np.float16)
        )
        gchunks = [_global_chunk(cls, s, j) for s in range(2) for j in range(4)]
        xq = np.concatenate([x[b][g * P : (g + 1) * P] for g in gchunks], axis=0)
        xqT = np.ascontiguousarray(xq.T.astype(np.float16))
        in_maps.append(
            {
                "xkvT": xkvT,
                "xqT": xqT,
                "wqT": wqT,
                "wkT": wkT,
                "wvT": wvT,
                "msk": masks[cls],
            }
        )

    res = run_bass_kernel_spmd(
        nc,
        in_maps,
        core_ids=list(range(8)),
        trace=bool(int(os.environ.get("KERNEL_TRACE", "0"))),
    )

    out = np.empty((B, T, D), np.float32)
    for c in range(8):
        b, cls = c // 2, c % 2
        o = res.results[c]["out"]
        for li, g in enumerate(
            _global_chunk(cls, s, j) for s in range(2) for j in range(4)
        ):
            out[b, g * P : (g + 1) * P] = o[li * P : (li + 1) * P]
    kernel._last_results = res
    return out


# revision 26
# speedup vs baseline: 1.1523x; 1.1523x over previous
"""Causal self-attention (B=4, T=2048, D=1024, single head, no scaling) on 8
Trainium2 NeuronCores.

Sharding: core c -> (batch b = c // 2, half h = c % 2).
The pair (2b, 2b+1) shares batch b:
  - K/V projections are split by key halves: core h projects keys
    [h*1024, (h+1)*1024) only, then the halves are exchanged with a
    2-core AllGather (HBM->HBM), eliminating the duplicated projection
    work of the pure batch-sharded layout.
  - Queries are split by interleaved 128-row chunks: core h owns global
    chunks {2j + h}. Slot L = chunks {h, 2+h, 4+h, 6+h} (kc extent 8),
    slot H = {8+h, ...} (extent 16) -- both classes run the same uniform
    program; validity lives in additive bf16 masks (0 / -30000).
  - The interleaving makes per-128-row causal extents nearly tight for
    BOTH classes, so the PV accumulation is trimmed per query group:
    extents {2,4,6,8} (L) and {10,12,14,16} (H) instead of {8,16}.

All matmul inputs are 16-bit (fp16 except exp(S) which needs bf16 range);
PSUM accumulation is fp32. 16-bit stationary halves the LDWEIGHTS pipe
(2 cycles/row vs fp32r's 4) which otherwise throttles the PE.

DMA ring assignment (FIFO per ring -- a gated descriptor blocks the ring
behind it):
  scalar ring: masks + wq + xq prefetch at t=0 (Q inputs must not queue
    behind the V bounce), then the lrow bounce chains, then out rows.
  sync ring:   wk/xkv/wv input streams, then the gather reloads (kt after
    the K AllGather completes, vsb after V's).
  gpsimd ring: bounce writes + the two AllGathers (keeps CC traffic off
    the input rings).
Row sums are emitted per slot as soon as that slot's last exp unit is
done, so linv[L] is ready before PV's first scale.
"""

import os
import numpy as np

import concourse.bass as bass
import concourse.mybir as mybir
import concourse.tile as tile
from concourse import bacc
from concourse.bass_utils import run_bass_kernel_spmd

B, T, D = 4, 2048, 1024
P = 128
NDC = D // P  # 8 contraction chunks over d_model
KH = T // 2  # 1024 keys per core (my half)
QS = 512  # query slot width
NQSUB = QS // P  # 4
SLOT_EXT = (8, 16)  # S key-chunk extent of slot L / slot H
PV_EXT = ((2, 4, 6, 8), (10, 12, 14, 16))  # trimmed PV extents per qsub
NMASK = 16  # mask units: L kc 0..7  +  H kc 8..15
MASK_VAL = -30000.0  # exactly representable in bf16; exp(S+MASK_VAL) == 0
RG = [[0, 1], [2, 3], [4, 5], [6, 7]]  # batch pairs

F32 = mybir.dt.float32
F32R = mybir.dt.float32r
BF16 = mybir.dt.bfloat16
F16 = mybir.dt.float16


def build_nc():
    nc = bacc.Bacc("TRN2", target_bir_lowering=False, debug=False, num_devices=8)

    # x[b].T columns of MY key half / MY query rows (local chunk order)
    xkvT = nc.dram_tensor("xkvT", [D, KH], F16, kind="ExternalInput")
    xqT = nc.dram_tensor("xqT", [D, 2 * QS], F16, kind="ExternalInput")
    wqT = nc.dram_tensor("wqT", [D, D], F16, kind="ExternalInput")  # Wq.T
    wkT = nc.dram_tensor("wkT", [D, D], F16, kind="ExternalInput")
    wvT = nc.dram_tensor("wvT", [D, D], F16, kind="ExternalInput")
    msk = nc.dram_tensor("msk", [NMASK, P, QS], BF16, kind="ExternalInput")
    out = nc.dram_tensor("out", [2 * QS, D], F32, kind="ExternalOutput")

    # collective bounce buffers. K^T half: rows e, cols k (local). V half:
    # rows t (local), cols e.
    kbnc = nc.dram_tensor("kbnc", [D, KH], F16, kind="Internal")
    vbnc = nc.dram_tensor("vbnc", [KH, D], BF16, kind="Internal")
    kg = nc.dram_tensor("kg", [2 * D, KH], F16, kind="Internal")
    vg = nc.dram_tensor("vg", [2 * KH, D], BF16, kind="Internal")

    xkvT_v = xkvT.rearrange("(c p) t -> p c t", p=P)
    xqT_v = xqT.rearrange("(c p) q -> p c q", p=P)
    w_v = {
        "q": wqT.rearrange("(c p) e -> p c e", p=P),
        "k": wkT.rearrange("(c p) e -> p c e", p=P),
        "v": wvT.rearrange("(c p) e -> p c e", p=P),
    }
    kbnc_v = kbnc.rearrange("(c p) k -> p c k", p=P)
    vbnc_v = vbnc.rearrange("(c p) e -> p c e", p=P)
    kg_v = kg.rearrange("(h c p) k -> p h c k", h=2, p=P)
    vg_v = vg.rearrange("(h c p) e -> p h c e", h=2, p=P)

    with tile.TileContext(nc) as tc:
        with (
            tc.tile_pool(name="persist", bufs=1) as persist,
            tc.tile_pool(name="small", bufs=2) as smallp,
            tc.tile_pool(name="dram", bufs=1, space="DRAM") as dramp,
        ):
            pT = persist.tile([P, 24, QS], BF16, tag="pT")  # exp(S^T)  24 KB/p
            ones_f32 = persist.tile([P, 1], F32, tag="ones_f32")
            nc.vector.memset(ones_f32, 1.0)
            ones = persist.tile([P, 1], F32R, tag="ones")
            nc.vector.tensor_copy(out=ones, in_=ones_f32)
            ones_bf = persist.tile([P, 1], BF16, tag="ones_bf")
            nc.vector.tensor_copy(out=ones_bf, in_=ones_f32)
            # exp bias: global constant -C (cancels in the l-normalization)
            negc = persist.tile([P, 1], F32, tag="negc")
            nc.vector.memset(negc, -8.0)
            linv = persist.tile([P, 2, NQSUB], F32, tag="linv")  # 1/l per slot
            warm = persist.tile([P, 2], F32R, tag="warm")
            nc.vector.tensor_copy(out=warm, in_=ones_f32.to_broadcast((P, 2)))

            # HAM warm-up: keep the PE busy while startup DMAs stream in.
            with tc.tile_pool(name="warmps", bufs=1, space="PSUM") as warmps:
                wps = warmps.tile([1, 2], F32)
                for wi in range(56):
                    nc.tensor.matmul(
                        wps, ones, warm, start=(wi == 0), stop=(wi == 55)
                    )

            # ---- scalar-ring prefetch: masks, wq, xq (needed by Q/S) ------
            maskp = tc.alloc_tile_pool(name="maskp", bufs=NMASK)
            mtiles = []
            for u in range(NMASK):
                mt = maskp.tile([P, QS], BF16, tag="mask", name=f"m{u}")
                nc.scalar.dma_start(out=mt, in_=msk[u, :, :])
                mtiles.append(mt)
            # allocated before wqp/xqp so pool releases stay LIFO
            qTp = tc.alloc_tile_pool(name="qTp", bufs=1)
            qT = qTp.tile([P, NDC, 2 * QS], F16, tag="qT")  # 16 KB/p
            wqp = tc.alloc_tile_pool(name="wqp", bufs=NDC)
            xqp = tc.alloc_tile_pool(name="xqp", bufs=2 * NDC)
            wqc = []
            xqc = {}
            for dc in range(NDC):
                w_t = wqp.tile([P, D], F16, tag="wqc", name=f"wq_{dc}")
                nc.scalar.dma_start(out=w_t, in_=w_v["q"][:, dc, :])
                wqc.append(w_t)
            for qs in range(2):
                for dc in range(NDC):
                    x_t = xqp.tile([P, QS], F16, tag="xq", name=f"xq_{dc}_{qs}")
                    nc.scalar.dma_start(
                        out=x_t, in_=xqT_v[:, dc, qs * QS : (qs + 1) * QS]
                    )
                    xqc[(dc, qs)] = x_t

            # xkv tiles live through K and V projections (stationary in V).
            xkvp = tc.alloc_tile_pool(name="xkvp", bufs=16)
            xkv = {}  # (dc, ts) -> tile [P, QS]

            # ============ Phase K: K^T[e, k] for my key half ================
            with (
                tc.tile_pool(name="wkp", bufs=8) as wkp,
                tc.tile_pool(name="kminep", bufs=1) as kminep,
                tc.tile_pool(name="kps", bufs=8, space="PSUM") as kpsp,
            ):
                kmine = kminep.tile([P, NDC, KH], F16, tag="kmine")  # 16 KB/p
                wkc = []
                # first-use order: (wk chunk, x chunk) per dc for slice 0
                for dc in range(NDC):
                    w_t = wkp.tile([P, D], F16, tag="wkc", name=f"wk_{dc}")
                    nc.sync.dma_start(out=w_t, in_=w_v["k"][:, dc, :])
                    wkc.append(w_t)
                    x_t = xkvp.tile([P, QS], F16, tag="xkv", name=f"xkv_{dc}_0")
                    nc.sync.dma_start(out=x_t, in_=xkvT_v[:, dc, 0:QS])
                    xkv[(dc, 0)] = x_t

                for ts in range(2):
                    if ts == 1:
                        for dc in range(NDC):
                            x_t = xkvp.tile(
                                [P, QS], F16, tag="xkv", name=f"xkv_{dc}_1"
                            )
                            nc.sync.dma_start(
                                out=x_t, in_=xkvT_v[:, dc, QS : 2 * QS]
                            )
                            xkv[(dc, 1)] = x_t
                    for half in range(2):
                        pss = [
                            kpsp.tile([P, QS], F32, tag="kps", name=f"kps_{ts}_{half}_{i}")
                            for i in range(4)
                        ]
                        for dc in range(NDC):
                            for ei, ec in enumerate(range(half * 4, half * 4 + 4)):
                                nc.tensor.matmul(
                                    pss[ei],
                                    wkc[dc][:, ec * P : (ec + 1) * P],
                                    xkv[(dc, ts)],
                                    start=(dc == 0),
                                    stop=(dc == NDC - 1),
                                )
                        for ei, ec in enumerate(range(half * 4, half * 4 + 4)):
                            nc.any.tensor_copy(
                                out=kmine[:, ec, ts * QS : (ts + 1) * QS],
                                in_=pss[ei],
                            )

                # bounce my K^T half to DRAM and gather the pair's full K^T
                for ec in range(NDC):
                    nc.gpsimd.dma_start(out=kbnc_v[:, ec, :], in_=kmine[:, ec, :])
                nc.gpsimd.collective_compute(
                    "AllGather",
                    mybir.AluOpType.bypass,
                    replica_groups=RG,
                    ins=[kbnc[:, :]],
                    outs=[kg[:, :]],
                )

            # ============ Phase V: V[t, e] for my key half ==================
            with (
                tc.tile_pool(name="wvp", bufs=8) as wvp,
                tc.tile_pool(name="vminep", bufs=1) as vminep,
                tc.tile_pool(name="vps", bufs=4, space="PSUM") as vpsp,
            ):
                vmine = vminep.tile([P, NDC, D], BF16, tag="vmine")  # 16 KB/p
                wvc = []
                for dc in range(NDC):
                    w_t = wvp.tile([P, D], F16, tag="wvc", name=f"wv_{dc}")
                    nc.sync.dma_start(out=w_t, in_=w_v["v"][:, dc, :])
                    wvc.append(w_t)
                for tc2 in range(NDC):  # my 8 key chunks of 128
                    ts, sub = tc2 // 4, tc2 % 4
                    for es in range(2):
                        ps = vpsp.tile([P, QS], F32, tag="vps")
                        for dc in range(NDC):
                            nc.tensor.matmul(
                                ps,
                                xkv[(dc, ts)][:, sub * P : (sub + 1) * P],
                                wvc[dc][:, es * QS : (es + 1) * QS],
                                start=(dc == 0),
                                stop=(dc == NDC - 1),
                            )
                        nc.any.tensor_copy(
                            out=vmine[:, tc2, es * QS : (es + 1) * QS], in_=ps
                        )
                for tc2 in range(NDC):
                    nc.gpsimd.dma_start(out=vbnc_v[:, tc2, :], in_=vmine[:, tc2, :])
                nc.gpsimd.collective_compute(
                    "AllGather",
                    mybir.AluOpType.bypass,
                    replica_groups=RG,
                    ins=[vbnc[:, :]],
                    outs=[vg[:, :]],
                )

            xkvp.release()

            # ============ Phase Q: Q^T[e, q] (inputs prefetched) ============
            with tc.tile_pool(name="qps", bufs=8, space="PSUM") as qpsp:
                for qs in range(2):
                    for half in range(2):
                        pss = [
                            qpsp.tile([P, QS], F32, tag="qps", name=f"qps_{qs}_{half}_{i}")
                            for i in range(4)
                        ]
                        for dc in range(NDC):
                            for ei, ec in enumerate(range(half * 4, half * 4 + 4)):
                                nc.tensor.matmul(
                                    pss[ei],
                                    wqc[dc][:, ec * P : (ec + 1) * P],
                                    xqc[(dc, qs)],
                                    start=(dc == 0),
                                    stop=(dc == NDC - 1),
                                )
                        for ei, ec in enumerate(range(half * 4, half * 4 + 4)):
                            nc.any.tensor_copy(
                                out=qT[:, ec, qs * QS : (qs + 1) * QS], in_=pss[ei]
                            )
            xqp.release()
            wqp.release()

            # ============ Phase S: S^T[k, q] + exp, from gathered K =========
            lrow_d = dramp.tile([2, QS], F32)
            with (
                tc.tile_pool(name="ktp", bufs=4) as ktp,
                tc.tile_pool(name="sps", bufs=6, space="PSUM") as spsp,
                tc.tile_pool(name="lrowp", bufs=2, space="PSUM") as lrowp,
            ):
                for kcg in range(16):  # global key chunk
                    h, kcl = kcg // 8, kcg % 8
                    kt = ktp.tile([P, NDC, P], F16, tag="kt")
                    nc.sync.dma_start(
                        out=kt, in_=kg_v[:, h, :, kcl * P : (kcl + 1) * P]
                    )
                    for slot in range(2):
                        if kcg >= SLOT_EXT[slot]:
                            continue
                        u = kcg if slot == 0 else 8 + kcg
                        sps = spsp.tile([P, QS], F32, tag="sps")
                        for ec in range(NDC):
                            nc.tensor.matmul(
                                sps,
                                kt[:, ec, :],
                                qT[:, ec, slot * QS : (slot + 1) * QS],
                                start=(ec == 0),
                                stop=(ec == NDC - 1),
                            )
                        # mask: L -> msk[kc] 0..7, H -> msk[kc] 8..15;
                        # H kc 0..7 is fully valid for both classes.
                        if (slot == 0) or (kcg >= 8):
                            nc.vector.tensor_add(out=sps, in0=sps, in1=mtiles[kcg])
                        nc.scalar.activation(
                            out=pT[:, u, :],
                            in_=sps,
                            func=mybir.ActivationFunctionType.Exp,
                            bias=negc[:, :],
                        )

                    # Row sums per slot as soon as its last unit is done:
                    # l[slot, q] = sum_k exp(S^T)[k, q]; DRAM-bounce transpose
                    # to column form, reciprocal overlaps the remaining S/PV.
                    for slot in range(2):
                        if kcg != SLOT_EXT[slot] - 1:
                            continue
                        ext = SLOT_EXT[slot]
                        lrow_ps = lrowp.tile(
                            [1, QS], F32, tag="lrow", name=f"lrow_{slot}"
                        )
                        for kc in range(ext):
                            u = kc if slot == 0 else 8 + kc
                            nc.tensor.matmul(
                                lrow_ps,
                                ones_bf,
                                pT[:, u, :],
                                start=(kc == 0),
                                stop=(kc == ext - 1),
                            )
                        lrow_sb = smallp.tile([1, QS], F32, tag="lrow_sb")
                        nc.any.tensor_copy(out=lrow_sb, in_=lrow_ps)
                        # DRAM APs must stay 2-D (1-D APs break NEFF load)
                        nc.scalar.dma_start(
                            out=lrow_d[slot : slot + 1, :], in_=lrow_sb[0:1, :]
                        )
                        l_col = smallp.tile([P, NQSUB], F32, tag="lcol")
                        nc.scalar.dma_start(
                            out=l_col,
                            in_=lrow_d[slot, :].rearrange("(q p) -> p q", p=P),
                        )
                        nc.vector.reciprocal(out=linv[:, slot, :], in_=l_col)

            qTp.release()
            maskp.release()

            # ============ Phase PV: O[q, e] from gathered V, trimmed ========
            with (
                tc.tile_pool(name="vsbp", bufs=1) as vsbp,
                tc.tile_pool(name="ostage", bufs=4) as ostagep,
                tc.tile_pool(name="ops", bufs=3, space="PSUM") as opsp,
            ):
                vsb = vsbp.tile([P, 16, D], BF16, tag="vsb")  # 32 KB/p
                for kcg in range(16):
                    h, tcl = kcg // 8, kcg % 8
                    nc.sync.dma_start(
                        out=vsb[:, kcg, :], in_=vg_v[:, h, tcl, :]
                    )

                for slot in range(2):
                    for qsub in range(NQSUB):
                        ext = PV_EXT[slot][qsub]
                        ops = opsp.tile([P, D], F32, tag="o")
                        for kc in range(ext):
                            u = kc if slot == 0 else 8 + kc
                            lhsT = pT[:, u, qsub * P : (qsub + 1) * P]
                            for es in range(2):
                                nc.tensor.matmul(
                                    ops[:, es * QS : (es + 1) * QS],
                                    lhsT,
                                    vsb[:, kc, es * QS : (es + 1) * QS],
                                    start=(kc == 0),
                                    stop=(kc == ext - 1),
                                )
                        o_sb = ostagep.tile([P, D], F32, tag="osb")
                        nc.vector.tensor_scalar_mul(
                            out=o_sb, in0=ops, scalar1=linv[:, slot, qsub : qsub + 1]
                        )
                        r0 = (slot * NQSUB + qsub) * P
                        nc.scalar.dma_start(out=out[r0 : r0 + P, :], in_=o_sb)

    nc.compile()
    return nc


_NC_CACHE = []


def _get_nc():
    if not _NC_CACHE:
        _NC_CACHE.append(build_nc())
    return _NC_CACHE[0]


def _global_chunk(cls, slot, j):
    """Global 128-row query chunk for class cls, slot, local chunk j."""
    return slot * 8 + 2 * j + cls


def _build_masks():
    """mask[u, k, q] additive (0 valid / MASK_VAL invalid) per class, bf16.

    Unit u = kc for slot L (0..7), 8 + kc for slot H (8..15).  Query column
    c = j*128 + i maps to global query (slot*8 + 2j + cls)*128 + i.
    """
    import ml_dtypes

    masks = []
    cols = np.arange(QS)
    j, i = cols // P, cols % P
    for cls in range(2):
        m = np.zeros((NMASK, P, QS), np.float32)
        for u in range(NMASK):
            slot = 0 if u < 8 else 1
            kglob = u * P + np.arange(P)[:, None]
            qglob = (_global_chunk(cls, slot, j) * P + i)[None, :]
            m[u] = np.where(kglob <= qglob, 0.0, MASK_VAL)
        masks.append(m.astype(ml_dtypes.bfloat16))
    return masks


def kernel(x, Wq, Wk, Wv):
    x = np.ascontiguousarray(np.asarray(x), dtype=np.float32)
    nc = _get_nc()
    masks = _build_masks()
    wqT = np.ascontiguousarray(np.asarray(Wq, np.float32).T.astype(np.float16))
    wkT = np.ascontiguousarray(np.asarray(Wk, np.float32).T.astype(np.float16))
    wvT = np.ascontiguousarray(np.asarray(Wv, np.float32).T.astype(np.float16))

    in_maps = []
    for c in range(8):
        b, cls = c // 2, c % 2
        xkvT = np.ascontiguousarray(
            x[b][cls * KH : (cls + 1) * KH].T.astype(np.float16)
        )
        gchunks = [_global_chunk(cls, s, j) for s in range(2) for j in range(4)]
        xq = np.concatenate([x[b][g * P : (g + 1) * P] for g in gchunks], axis=0)
        xqT = np.ascontiguousarray(xq.T.astype(np.float16))
        in_maps.append(
            {
                "xkvT": xkvT,
                "xqT": xqT,
                "wqT": wqT,
                "wkT": wkT,
                "wvT": wvT,
                "msk": masks[cls],
            }
        )

    res = run_bass_kernel_spmd(
        nc,
        in_maps,
        core_ids=list(range(8)),
        trace=bool(int(os.environ.get("KERNEL_TRACE", "0"))),
    )

    out = np.empty((B, T, D), np.float32)
    for c in range(8):
        b, cls = c // 2, c % 2
        o = res.results[c]["out"]
        for li, g in enumerate(
            _global_chunk(cls, s, j) for s in range(2) for j in range(4)
        ):
            out[b, g * P : (g + 1) * P] = o[li * P : (li + 1) * P]
    kernel._last_results = res
    return out
